# revision 19
# baseline (speedup 1.0000x reference)
"""Trainium2 Bass kernel for nn_CaC_Module (dynamic-kernel dilated depthwise CNN).

Per-sample computation (b=8 sharded 1/core across 8 NeuronCores):
  query = Wq @ x          (1x1 conv, [9, hw])
  q     = softmax(query over hw)          (bq cancels in softmax -> ignored)
  kern  = Wk @ (x @ q^T) + bk             (associativity: avoids the big
                                           key GEMM entirely; bk folds in
                                           because sum_n q = 1)
  out   = x * sum_d sigmoid(depthwise3x3(x, kern, dil=d)),  d in (1,3,5)

Mapping (v4 - fp8 DoubleRow, DMA packet-size aware, engine-balanced):
  - Depthwise conv = fp8e4 DoubleRow matmuls: TWO diagonal stationaries
    diag(kern[:,ta]), diag(kern[:,tb]) stacked on the k-tile dim, with the
    moving operand an OVERLAPPING strided AP over a flat padded fp8 copy of
    x (row stride 69 = 64 data + 5 shared zero margin, 5 zero rows top and
    bottom): k-tile stride = tap-offset delta.  One matmul = two taps at
    one fp16-matmul cost -> 2x PE throughput on the conv (83% of PE work).
  - DMA descriptors are the bottleneck, not bandwidth: every transfer is
    laid out host-side so per-partition rows are 2KB+ contiguous (xc in
    column halves, xT8/wkT pre-arranged [128, ...] contiguous, one output
    transfer per window).  No inter-transfer deps: the 16 DMA engines
    drain the ring in issue order at full aggregate bandwidth.
  - The padded fp8 conv buffer is NOT DMA'd: GpSimdE (idle in the head)
    casts it out of the fp16 image per column-half, cutting 2.6MB off the
    critical input stream.
  - Head pipeline: query GEMM (WqT stationary: 9-col ldweights) -> exp
    (bias -2 for the fp8e4 ceiling; cancels in softmax) -> per-chunk PE
    transpose -> fp8 qT -> G-GEMM chunk-pair (fp8 DoubleRow) accumulates
    immediately.  Softmax denominator = PE ones-reduce of the SAME fp8 e
    values, so quantization partially cancels in q = e/sum(e).
  - All four channel-blocks' diagonal stationaries are built upfront so
    the PE never idles >3.4us at block bounds (HAM re-throttle to 1.2GHz).
  - Center tap (same offset for all 3 dilations) on VectorE as an fp16
    STT that reads the conv PSUM and writes z to SBUF fp16: frees the
    PSUM bank early and lets sigmoid read SBUF at 2 elem/cycle.  Adds +
    final x*w on GpSimdE; output DMA'd as fp16.
"""
import numpy as np

C, H, W = 512, 64, 64
HW = H * W                # 4096 (compact n-space)
P, CB = 128, 4
RS = 69                   # padded row stride: 64 data + 5 shared zero margin
HEAD = 5                  # guard zeros before row 0
VPAD = 5                  # zero rows above/below the image
XLEN = 5120               # per-channel flat fp8 buffer length
RT = 7                    # image rows per conv window
NW = 10                   # 9 windows x 7 rows + 1 window x 1 row = 64 rows
NCH = 32                  # compact n-chunks of 128
RATES = (1, 3, 5)
PAIRS = ((0, 1), (2, 3), (5, 6), (7, 8))  # adjacent tap pairs; center=4 on DVE
NCORES = 8

_CACHE = {}


def _flat(r, x):
    # fp8 buffer index of image row r (may be in [-5, 69)), column x
    return HEAD + (VPAD + r) * RS + x


def _build_program():
    import concourse.bacc as bacc
    import concourse.mybir as mybir
    from concourse.tile import TileContext
    from concourse.ap import AP

    dt = mybir.dt
    AF = mybir.ActivationFunctionType
    ALU = mybir.AluOpType
    PM = mybir.MatmulPerfMode
    f32, f16, f8 = dt.float32, dt.float16, dt.float8e4

    nc = bacc.Bacc()
    xc_d = nc.declare_dram_parameter("xc", [C, HW], f16, isOutput=False)
    xT8_d = nc.declare_dram_parameter("xT8", [P, NCH * C], f8, isOutput=False)
    wkT_d = nc.declare_dram_parameter("wkT", [P, CB * C], f16, isOutput=False)
    wqT_d = nc.declare_dram_parameter("wqT", [P, CB * 9], f16, isOutput=False)
    bk_d = nc.declare_dram_parameter("bk", [P, CB], f32, isOutput=False)
    id9h_d = nc.declare_dram_parameter("id9h", [9, 9], f16, isOutput=False)
    id128_d = nc.declare_dram_parameter("id128", [P, P], f16, isOutput=False)
    ones8_d = nc.declare_dram_parameter("ones8", [P, 1], f8, isOutput=False)
    out_d = nc.declare_dram_parameter("out", [C, HW], f16, isOutput=True)

    def winsize(w):
        return (RT * RS) if w < NW - 1 else RS  # 483 or 69

    def nrows(w):
        return RT if w < NW - 1 else 1

    with TileContext(nc) as tc:
        with (
            tc.tile_pool(name="const", bufs=1) as cpool,
            tc.tile_pool(name="sigp", bufs=6) as sigp,
            tc.tile_pool(name="zp", bufs=6) as zp,
            tc.tile_pool(name="tmpp", bufs=3) as tmpp,
            tc.tile_pool(name="outp", bufs=3) as opool,
            tc.tile_pool(name="ps", bufs=7, space="PSUM") as ps,
            tc.tile_pool(name="gps", bufs=1, space="PSUM") as gpsp,
        ):
            xc = cpool.tile([P, CB, HW], f16)
            xf8 = cpool.tile([P, CB, XLEN], f8)
            xT8 = cpool.tile([P, NCH, C], f8)
            wkT = cpool.tile([P, CB, C], f16)
            wqT = cpool.tile([P, CB, 9], f16)
            bk = cpool.tile([P, CB], f32)
            id9h = cpool.tile([9, 9], f16)
            id128 = cpool.tile([P, P], f16)
            ones8 = cpool.tile([P, 1], f8)
            qbuf = cpool.tile([32, HW], f16)  # rows 0:9 hold exp(query-2)
            qT8 = cpool.tile([P, NCH, 16], f8)
            bm2 = cpool.tile([P, 1], f32)
            rinv = cpool.tile([9, 1], f32)
            gs = cpool.tile([9, C], f16)
            G = cpool.tile([P, CB, 9], f16)
            kern = cpool.tile([P, CB, 9], f32)
            diag8 = cpool.tile([P, CB, 9, P], f8)

            # ---- input DMAs: no inter-transfer deps; 2KB+ contiguous
            # per-partition rows so the transfers are bandwidth- not
            # descriptor-bound.  Priority: consts, xc half 0, xT8 half 0,
            # xc half 1, xT8 half 1, wkT ----
            nc.sync.dma_start(
                out=wqT[:], in_=wqT_d[:].rearrange("p (cb t) -> p cb t", t=9))
            nc.sync.dma_start(out=id9h[:], in_=id9h_d[:])
            nc.sync.dma_start(out=id128[:], in_=id128_d[:])
            nc.sync.dma_start(out=ones8[:], in_=ones8_d[:])
            nc.sync.dma_start(out=bk[:], in_=bk_d[:])
            nc.vector.memset(bm2[:], -2.0)
            for h in range(2):
                a, b = h * 2048, (h + 1) * 2048
                for cb in range(CB):
                    nc.sync.dma_start(
                        out=xc[:, cb, a:b], in_=xc_d[cb * P:(cb + 1) * P, a:b])
                nc.sync.dma_start(
                    out=xT8[:, 16 * h:16 * h + 16],
                    in_=xT8_d[:, 8192 * h:8192 * (h + 1)]
                    .rearrange("p (n c) -> p n c", c=C))
            nc.sync.dma_start(
                out=wkT[:], in_=wkT_d[:].rearrange("p (cb o) -> p cb o", o=C))

            # ---- padded fp8 conv buffer built on GpSimd (idle in the
            # head): zero the margins, then cast the image in per
            # column-half as xc lands ----
            for cb in range(CB):
                nc.vector.memset(xf8[:, cb, 0:_flat(0, 0)], 0.0)
                nc.vector.memset(xf8[:, cb, _flat(H, 0):XLEN], 0.0)
                nc.vector.memset(
                    xf8[:, cb, _flat(0, 0):_flat(0, 0) + H * RS]
                    .rearrange("p (r c) -> p r c", c=RS)[:, :, W:RS], 0.0)
            for h in range(2):
                for cb in range(CB):
                    nc.vector.tensor_copy(
                        xf8[:, cb, _flat(32 * h, 0):_flat(32 * h, 0) + 32 * RS]
                        .rearrange("p (r c) -> p r c", c=RS)[:, :, 0:W],
                        xc[:, cb, 2048 * h:2048 * (h + 1)]
                        .rearrange("p (r c) -> p r c", c=W))

            # ---- PE warmup: ~3.4us of dummy matmuls on the identity tile
            # so the HAM clock-gate opens (1.2 -> 2.4 GHz) before the
            # first real matmul ----
            pw = ps.tile([P, P], f32, tag="ps")
            for i in range(32):
                nc.tensor.matmul(pw[:], lhsT=id128[:], rhs=id128[:],
                                 start=(i == 0), stop=(i == 31))

            # ---- head pipeline per 512-col window: query GEMM -> exp ->
            # per-chunk transpose -> fp8 qT -> G-GEMM chunk pair ----
            pgt = gpsp.tile([9, C], f32, tag="gps")
            for w in range(8):
                psq = ps.tile([9, 512], f32, tag="ps")
                for kc in range(CB):
                    nc.tensor.matmul(
                        psq[:], lhsT=wqT[:, kc],
                        rhs=xc[:, kc, w * 512:(w + 1) * 512],
                        start=(kc == 0), stop=(kc == CB - 1))
                nc.scalar.activation(
                    qbuf[0:9, w * 512:(w + 1) * 512], psq[:],
                    AF.Exp, bias=bm2[0:9])
                for k in range(4 * w, 4 * w + 4):
                    pst = ps.tile([P, 9], f16, tag="ps")
                    nc.tensor.transpose(
                        pst[:], qbuf[0:9, k * P:(k + 1) * P], id9h[:])
                    nc.vector.tensor_copy(qT8[:, k, 0:9], pst[:])
                    if k % 2 == 1:
                        i = k // 2
                        nc.tensor.matmul(
                            pgt[:],
                            lhsT=qT8[:, 2 * i:2 * i + 2, 0:9],
                            rhs=xT8[:, 2 * i:2 * i + 2, :],
                            start=(i == 0), stop=(i == NCH // 2 - 1),
                            perf_mode=PM.DoubleRow, skip_group_check=True)

            # ---- softmax denominator: PE ones-reduce of the SAME fp8 e
            # values (quantization partially cancels in e/sum) ----
            sps = ps.tile([9, 1], f32, tag="ps")
            for k in range(NCH):
                nc.tensor.matmul(
                    sps[:], lhsT=qT8[:, k, 0:9], rhs=ones8[:],
                    start=(k == 0), stop=(k == NCH - 1))
            nc.vector.reciprocal(rinv[:], sps[:])
            nc.vector.tensor_scalar_mul(gs[:], pgt[:], rinv[:])

            # ---- G[ci, t] = gs^T ----
            for ci in range(CB):
                psx = ps.tile([P, 9], f16, tag="ps")
                nc.tensor.transpose(
                    psx[:], gs[:, ci * P:(ci + 1) * P], id9h[:])
                nc.vector.tensor_copy(G[:, ci], psx[:])

            # ---- kern[c, t] = sum_ci Wk[c,ci] G[ci,t] + bk[c]; the
            # diagonal stationaries for ALL channel blocks are built right
            # behind it so the conv never stalls on DVE at cb bounds ----
            for co in range(CB):
                psn = ps.tile([P, 9], f32, tag="ps")
                for ci in range(CB):
                    nc.tensor.matmul(
                        psn[:],
                        lhsT=wkT[:, ci, co * P:(co + 1) * P],
                        rhs=G[:, ci],
                        start=(ci == 0), stop=(ci == CB - 1))
                nc.vector.tensor_scalar_add(kern[:, co], psn[:], bk[:, co:co + 1])
                for t in range(9):
                    nc.vector.tensor_scalar_mul(
                        diag8[:, co, t], id128[:], kern[:, co, t:t + 1])

            # ---- depthwise convs: fp8 DoubleRow tap-pairs on PE; center
            # tap as fp16 STT on DVE draining PSUM+center -> z in SBUF;
            # sigmoid from SBUF right behind it; adds + x*w on GpSimd ----
            for cb in range(CB):
                for w in range(NW):
                    N = winsize(w)
                    nr = nrows(w)
                    r0 = RT * w
                    # keep the last windows PE-only: shortens the
                    # end-of-kernel drain chain
                    pe_only = (cb == CB - 1 and w >= NW - 2)
                    s = []
                    for di, d in enumerate(RATES):
                        pd = ps.tile([P, N], f32, tag="ps")
                        offs = {t: _flat(r0 + (t // 3 - 1) * d, (t % 3 - 1) * d)
                                for t in range(9)}
                        for i, (ta, tb) in enumerate(PAIRS):
                            base = xf8[:, cb, offs[ta]:offs[ta] + N]
                            mv = AP(base.tensor, base.offset,
                                    [list(base.ap[0]),
                                     [offs[tb] - offs[ta], 2], [1, N]])
                            nc.tensor.matmul(
                                pd[:], lhsT=diag8[:, cb, ta:ta + 2], rhs=mv,
                                start=(i == 0),
                                stop=(i == len(PAIRS) - 1 and not pe_only),
                                perf_mode=PM.DoubleRow)
                        pdv = pd[:].rearrange("p (r c) -> p r c", c=RS)[:, :, 0:W]
                        xcv = (xc[:, cb, r0 * W:(r0 + nr) * W]
                               .rearrange("p (r c) -> p r c", c=W))
                        st = sigp.tile([P, RT * W], f16, tag="sig")
                        stv = st[:, 0:nr * W].rearrange("p (r c) -> p r c", c=W)
                        if pe_only:
                            nc.tensor.matmul(
                                pd[:], lhsT=diag8[:, cb, 4],
                                rhs=xf8[:, cb, offs[4]:offs[4] + N],
                                start=False, stop=True)
                            nc.scalar.activation(stv, pdv, AF.Sigmoid)
                        else:
                            z = zp.tile([P, RT * W], f16, tag="z")
                            zv = z[:, 0:nr * W].rearrange("p (r c) -> p r c", c=W)
                            nc.vector.scalar_tensor_tensor(
                                zv, in0=xcv, scalar=kern[:, cb, 4:5],
                                in1=pdv, op0=ALU.mult, op1=ALU.add)
                            nc.scalar.activation(
                                st[:, 0:nr * W], z[:, 0:nr * W], AF.Sigmoid)
                        s.append(st)
                    t01 = tmpp.tile([P, RT * W], f16, tag="t01")
                    w3 = tmpp.tile([P, RT * W], f16, tag="w3")
                    nc.gpsimd.tensor_add(
                        t01[:, 0:nr * W], s[0][:, 0:nr * W], s[1][:, 0:nr * W])
                    nc.gpsimd.tensor_add(
                        w3[:, 0:nr * W], t01[:, 0:nr * W], s[2][:, 0:nr * W])
                    ot = opool.tile([P, RT * W], f16, tag="ot")
                    nc.gpsimd.tensor_mul(
                        ot[:, 0:nr * W], w3[:, 0:nr * W],
                        xc[:, cb, r0 * W:(r0 + nr) * W])
                    nc.sync.dma_start(
                        out=out_d[cb * P:(cb + 1) * P, r0 * W:(r0 + nr) * W],
                        in_=ot[:, 0:nr * W])
    nc.finalize()
    return nc


def _get_program():
    if "nc" not in _CACHE:
        _CACHE["nc"] = _build_program()
    return _CACHE["nc"]


def make_in_maps(x, Wk, bk, Wq, bq=None):
    import ml_dtypes
    E4 = ml_dtypes.float8_e4m3

    x = np.ascontiguousarray(np.asarray(x, dtype=np.float32))
    B = x.shape[0]
    assert B == NCORES and x.shape[1:] == (C, H, W)
    x16 = x.astype(np.float16)
    xc = np.ascontiguousarray(x16.reshape(B, C, HW))
    # compact transposed fp8 for the G GEMM, pre-arranged so SBUF
    # partition p's row is contiguous in DRAM: [128, NCH*C]
    xT8 = np.swapaxes(xc.astype(E4), 1, 2)          # [B, HW, C]
    xT8 = xT8.reshape(B, NCH, P, C).transpose(0, 2, 1, 3)  # [B, P, NCH, C]
    xT8 = np.ascontiguousarray(xT8.reshape(B, P, NCH * C))
    wkT = np.asarray(Wk, np.float32).T.astype(np.float16)  # [C, C]
    wkT = wkT.reshape(CB, P, C).transpose(1, 0, 2).reshape(P, CB * C)
    wqT = np.asarray(Wq, np.float32).T.astype(np.float16)   # [C, 9]
    wqT = wqT.reshape(CB, P, 9).transpose(1, 0, 2).reshape(P, CB * 9)
    bkh = np.asarray(bk, np.float32).reshape(CB, P).T       # [P, CB]
    shared = {
        "wkT": np.ascontiguousarray(wkT),
        "wqT": np.ascontiguousarray(wqT),
        "bk": np.ascontiguousarray(bkh),
        "id9h": np.eye(9, dtype=np.float16),
        "id128": np.eye(P, dtype=np.float16),
        "ones8": np.ones((P, 1), dtype=E4),
    }
    return [dict(shared, xc=xc[i], xT8=xT8[i]) for i in range(B)]


def kernel(x, Wk, bk, Wq, bq):
    from concourse.bass_utils import run_bass_kernel_spmd

    in_maps = make_in_maps(x, Wk, bk, Wq, bq)
    nc = _get_program()
    res = run_bass_kernel_spmd(nc, in_maps, list(range(NCORES))).results
    return np.stack([res[i]["out"].reshape(C, H, W)
                     for i in range(NCORES)]).astype(np.float32)
